# revision 17
# baseline (speedup 1.0000x reference)
"""Trainium2 Bass kernel for nn_GPSModel (gnn_message_passing).

Strategy: data-parallel over graphs (2048 graphs -> 256/core on 8 cores).
Only the first 24 (real) nodes per graph matter: the dense pad region
(rows/cols 24..30) never reaches an output, and the pad mask never fires
for real nodes (z in 1..9).

Per core, per 4-graph block, layout [(g4, k32)=128 partitions]:
  - x[g,(i,j)] = mul[et]*dist + bias[et] computed on device from pos
    (et tables host-gathered from the tiny 100-entry params).
  - replicate x across the 32 k-partitions with a one-hot matmul,
  - ACT Square(scale=inv_k, bias=-mean_k*inv_k) then ACT Exp(scale=-0.5)
    -> raw gaussians gbf [(g,k), (i,j)] (the 1/(sqrt(2pi)std) norm is
    folded into the downstream weights on host),
  - node path: DVE reduce over j -> [(g,k), i], per-graph matmul vs
    Wc = diag(norm) @ W_edge_proj @ W_nodes, add host-gathered
    atom_emb@W_nodes rows -> node_feat,
  - edge path: GPSIMD ap_gather pulls the 288 per-edge pairs straight
    out of the per-graph [576]-entry table (device-local gather), then
    per-96-edge matmuls vs We2 = diag(norm) @ W_edges -> edge_feat.
"""

import numpy as np

G, MAXN, NPG, EPG = 2048, 30, 24, 288
K, EMB, DH, ATOM_TYPES = 32, 32, 64, 10
N = G * NPG
E = G * EPG
NCORES = 8
GC = G // NCORES          # graphs per core
BLK = 4                   # graphs per block
SB = 64                   # graphs per super-block (prep phase)

_PROG_CACHE = {}


# ---------------------------------------------------------------- reference
def _numpy_reference(z, pos, batch_mapping, edge_index, atom_emb, W_edge_proj,
                     b_edge_proj, gbf_means, gbf_stds, gbf_mul, gbf_bias,
                     W_nodes, b_nodes, W_edges, b_edges):
    """Pure-numpy port of the oracle; fallback when input assumptions fail."""
    n = z.shape[0]
    counts = np.bincount(batch_mapping, minlength=G).astype(np.int64)
    ptr = np.concatenate([[0], np.cumsum(counts)])[:-1]
    local = np.arange(n) - ptr[batch_mapping]
    z_dense = np.zeros((G, MAXN), z.dtype)
    z_dense[batch_mapping, local] = z
    pos_dense = np.zeros((G, MAXN, 3), pos.dtype)
    pos_dense[batch_mapping, local] = pos
    delta = pos_dense[:, None, :, :] - pos_dense[:, :, None, :]
    sq = np.sum(delta * delta, axis=-1)
    dist = np.where(sq == 0, 0.0, np.sqrt(np.where(sq == 0, 1.0, sq))).astype(np.float32)
    edge_type = z_dense[:, :, None] * ATOM_TYPES + z_dense[:, None, :]
    x = gbf_mul[edge_type] * dist + gbf_bias[edge_type]
    std = np.abs(gbf_stds) + 1e-5
    a = (x[..., None] - gbf_means) / std
    gbf_feature = np.exp(-0.5 * a * a) / (np.sqrt(2.0 * np.pi) * std)
    pad = (z_dense == 0)
    edge_features = np.where(pad[:, None, :, None], 0.0, gbf_feature).astype(np.float32)
    gnf = atom_emb[z_dense] + edge_features.sum(axis=2) @ W_edge_proj + b_edge_proj
    node_flat = gnf[batch_mapping, local]
    ei = edge_index[::-1]
    ebm = batch_mapping[ei[0]]
    sl = ei[0] - ptr[ebm]
    dl = ei[1] - ptr[ebm]
    edge_sel = edge_features[ebm, sl, dl, :]
    node_feat = (node_flat @ W_nodes + b_nodes).astype(np.float32)
    edge_feat = (edge_sel @ W_edges + b_edges).astype(np.float32)
    return node_feat, edge_feat


# ---------------------------------------------------------------- builder
def _build_program(gc):
    """Build the SPMD Bass program for `gc` graphs per core."""
    import concourse.bass as bass
    import concourse.bacc as bacc
    import concourse.mybir as mybir
    from concourse import tile

    f32 = mybir.dt.float32
    i16 = mybir.dt.int16
    Exp = mybir.ActivationFunctionType.Exp
    Square = mybir.ActivationFunctionType.Square
    Sqrt = mybir.ActivationFunctionType.Sqrt
    Alu = mybir.AluOpType
    AX = mybir.AxisListType

    nblk = gc // BLK
    nsb = (gc + SB - 1) // SB
    blk_per_sb = SB // BLK

    nc = bacc.Bacc("TRN2", target_bir_lowering=False, debug=False,
                   num_devices=NCORES)

    dt_pos = nc.dram_tensor("pos_in", [gc * NPG, 3], f32, kind="ExternalInput").ap()
    dt_mulT = nc.dram_tensor("mulT_in", [gc, 576], f32, kind="ExternalInput").ap()
    dt_biasT = nc.dram_tensor("biasT_in", [gc, 576], f32, kind="ExternalInput").ap()
    dt_aembR = nc.dram_tensor("aembR_in", [nblk, NPG, BLK * DH], f32, kind="ExternalInput").ap()
    dt_idx = nc.dram_tensor("idx_in", [nblk, 128, EPG // 16], i16, kind="ExternalInput").ap()
    dt_invc = nc.dram_tensor("invc_in", [128, 1], f32, kind="ExternalInput").ap()
    dt_nmic = nc.dram_tensor("nmic_in", [128, 1], f32, kind="ExternalInput").ap()
    dt_sel4 = nc.dram_tensor("sel4_in", [128, 8 * 128], f32, kind="ExternalInput").ap()
    dt_Wc = nc.dram_tensor("Wc_in", [128, BLK * DH], f32, kind="ExternalInput").ap()
    dt_We2 = nc.dram_tensor("We2_in", [128, BLK * DH], f32, kind="ExternalInput").ap()
    dt_biasE = nc.dram_tensor("biasE_in", [96, 12 * DH], f32, kind="ExternalInput").ap()

    dt_node = nc.dram_tensor("node_out", [gc * NPG, DH], f32, kind="ExternalOutput").ap()
    dt_edge = nc.dram_tensor("edge_out", [gc * EPG, DH], f32, kind="ExternalOutput").ap()

    with tile.TileContext(nc) as tc:
        with (
            tc.tile_pool(name="const", bufs=1) as cpool,
            tc.tile_pool(name="persist", bufs=1) as ppool,
            tc.tile_pool(name="work", bufs=3) as wpool,
            tc.tile_pool(name="out", bufs=3) as opool,
            tc.tile_pool(name="psx", bufs=1, space=bass.MemorySpace.PSUM) as psx,
            tc.tile_pool(name="psn", bufs=2, space=bass.MemorySpace.PSUM) as psn,
            tc.tile_pool(name="pse", bufs=2, space=bass.MemorySpace.PSUM) as pse,
        ):
            # ---- constants (loaded once) ----
            invc = cpool.tile([128, 1], f32, tag="invc")
            nmic = cpool.tile([128, 1], f32, tag="nmic")
            sel4 = cpool.tile([128, 8 * 128], f32, tag="sel4")
            Wc = cpool.tile([128, BLK * DH], f32, tag="Wc")
            We2 = cpool.tile([128, BLK * DH], f32, tag="We2")
            biasE = cpool.tile([96, 12 * DH], f32, tag="biasE")
            nc.sync.dma_start(invc[:], dt_invc)
            nc.sync.dma_start(nmic[:], dt_nmic)
            nc.sync.dma_start(sel4[:], dt_sel4)
            nc.sync.dma_start(Wc[:], dt_Wc)
            nc.sync.dma_start(We2[:], dt_We2)
            nc.sync.dma_start(biasE[:], dt_biasE)

            # ---- prep phase: x[g,(i,j)] per super-block ----
            x_tiles = []
            gs_of = []
            pos_r = dt_pos.rearrange("(g i) x -> g i x", i=NPG)
            for s in range(nsb):
                g0 = s * SB
                gs = min(SB, gc - g0)
                gs_of.append(gs)
                pos_t = ppool.tile([gs, NPG, 3], f32, tag=f"pos{s}")
                mulT_t = ppool.tile([gs, 576], f32, tag=f"mulT{s}")
                biasT_t = ppool.tile([gs, 576], f32, tag=f"biasT{s}")
                nc.sync.dma_start(pos_t[:], pos_r[g0:g0 + gs])
                nc.sync.dma_start(mulT_t[:], dt_mulT[g0:g0 + gs])
                nc.sync.dma_start(biasT_t[:], dt_biasT[g0:g0 + gs])

                delta = wpool.tile([gs, NPG, NPG, 3], f32, tag="delta")
                a_j = pos_t[:, None, :, :].broadcast_to((gs, NPG, NPG, 3))
                a_i = pos_t[:, :, None, :].broadcast_to((gs, NPG, NPG, 3))
                nc.vector.tensor_tensor(delta[:], a_j, a_i, Alu.subtract)
                sqf = wpool.tile([gs, NPG * NPG, 3], f32, tag="sqf")
                d2 = delta[:].rearrange("g i j x -> g (i j) x")
                nc.gpsimd.tensor_tensor(sqf[:], d2, d2, Alu.mult)
                sq = wpool.tile([gs, 576], f32, tag="sq")
                nc.vector.tensor_reduce(sq[:, :, None], sqf[:], axis=AX.X, op=Alu.add)
                dist = wpool.tile([gs, 576], f32, tag="dist")
                nc.scalar.activation(dist[:], sq[:], Sqrt)
                xt = ppool.tile([gs, 576], f32, tag=f"x{s}")
                nc.gpsimd.tensor_tensor(xt[:], dist[:], mulT_t[:], Alu.mult)
                nc.gpsimd.tensor_tensor(xt[:], xt[:], biasT_t[:], Alu.add)
                x_tiles.append(xt)

            # ---- aembR / idx loads per super-block ----
            aemb_tiles = []
            idx_tiles = []
            aemb_r = dt_aembR.rearrange("b i c -> i b c")
            idx_r = dt_idx.rearrange("b p s -> p b s")
            for s in range(nsb):
                b0 = s * blk_per_sb
                bs = min(blk_per_sb, nblk - b0)
                at = ppool.tile([NPG, bs, BLK * DH], f32, tag=f"aemb{s}")
                it = ppool.tile([128, bs, EPG // 16], i16, tag=f"idx{s}")
                nc.sync.dma_start(at[:], aemb_r[:, b0:b0 + bs])
                nc.sync.dma_start(it[:], idx_r[:, b0:b0 + bs])
                aemb_tiles.append(at)
                idx_tiles.append(it)

            # ---- block loop ----
            node_r = dt_node.rearrange("(b g i) f -> b i g f", g=BLK, i=NPG)
            edge_r = dt_edge.rearrange("(b gm p) f -> b p gm f", gm=BLK * 3, p=96)
            for b in range(nblk):
                s = b // blk_per_sb
                bi = b % blk_per_sb
                xt = x_tiles[s]
                g0 = b * BLK - s * SB   # graph offset within super-block

                # replicate the block's 4 x-rows over the 32 k-partitions.
                # Engine APs may only start at partition 0/32/64/96, so read
                # the 32-row quadrant and pick rows with a free-sliced one-hot.
                q0, r = 32 * (g0 // 32), (g0 % 32) // BLK
                ks = min(32, gs_of[s] - q0)
                xR = psx.tile([128, 576], f32, tag="xR")
                selq = sel4[q0:q0 + ks, 128 * r:128 * (r + 1)]
                nc.tensor.matmul(xR[:, 0:512], selq,
                                 xt[q0:q0 + ks, 0:512], start=True, stop=True)
                nc.tensor.matmul(xR[:, 512:576], selq,
                                 xt[q0:q0 + ks, 512:576], start=True, stop=True)

                # gaussians
                q = wpool.tile([128, 576], f32, tag="q")
                nc.scalar.activation(q[:], xR[:], Square,
                                     bias=nmic[:], scale=invc[:])
                gbf = wpool.tile([128, 576], f32, tag="gbf")
                nc.scalar.activation(gbf[:], q[:], Exp, scale=-0.5)

                # node path
                ns = wpool.tile([128, NPG], f32, tag="ns")
                nc.vector.tensor_reduce(
                    ns[:], gbf[:].rearrange("p (i j) -> p i j", i=NPG),
                    axis=AX.X, op=Alu.add)
                pn = psn.tile([NPG, BLK * DH], f32, tag="pn")
                nc.tensor.matmul(pn[:], ns[:], Wc[:], start=True, stop=True)
                nf = opool.tile([NPG, BLK * DH], f32, tag="nf")
                nc.vector.tensor_tensor(nf[:], pn[:], aemb_tiles[s][:, bi, :], Alu.add)
                nc.sync.dma_start(node_r[b], nf[:].rearrange("i (g f) -> i g f", g=BLK))

                # edge path
                gath = wpool.tile([128, EPG], f32, tag="gath")
                nc.gpsimd.ap_gather(gath[:], gbf[:], idx_tiles[s][:, bi, :],
                                    channels=128, num_elems=576, d=1, num_idxs=EPG)
                pe = pse.tile([96, 12 * DH], f32, tag="pe")
                for m in range(3):
                    nc.tensor.matmul(
                        pe[:, m * BLK * DH:(m + 1) * BLK * DH],
                        gath[:].rearrange("p (m e) -> p m e", m=3)[:, m, :],
                        We2[:], start=True, stop=True)
                ef = opool.tile([96, 12 * DH], f32, tag="ef")
                nc.vector.tensor_tensor(
                    ef[:].rearrange("p (g m f) -> p m g f", g=BLK, m=3),
                    pe[:].rearrange("p (m g f) -> p m g f", m=3, g=BLK),
                    biasE[:].rearrange("p (m g f) -> p m g f", m=3, g=BLK),
                    Alu.add)
                nc.sync.dma_start(
                    edge_r[b],
                    ef[:].rearrange("p (gm f) -> p gm f", gm=BLK * 3))

    nc.compile()
    return nc


# ---------------------------------------------------------------- host prep
def _host_prep(z, pos, edge_index, atom_emb, W_edge_proj, b_edge_proj,
               gbf_means, gbf_stds, gbf_mul, gbf_bias,
               W_nodes, b_nodes, W_edges, b_edges, gc):
    f32 = np.float32
    std = np.abs(gbf_stds.astype(np.float64)) + 1e-5
    inv = (1.0 / std).astype(f32)
    norm = (1.0 / (np.sqrt(2.0 * np.pi) * std)).astype(f32)

    Wc = (norm[:, None] * (W_edge_proj.astype(np.float64) @ W_nodes.astype(np.float64))).astype(f32)
    bc = (b_edge_proj @ W_nodes + b_nodes).astype(f32)
    We2 = (norm[:, None] * W_edges).astype(f32)

    zl = z.reshape(G, NPG).astype(np.int64)
    et = zl[:, :, None] * ATOM_TYPES + zl[:, None, :]      # [G,24,24]
    mulT = gbf_mul[et].astype(f32).reshape(G, 576)
    biasT = gbf_bias[et].astype(f32).reshape(G, 576)

    aembW = (atom_emb @ W_nodes + bc).astype(f32)          # [10,64]
    aemb = aembW[z]                                        # [N,64]
    nblk = gc // BLK
    aembR = (aemb.reshape(NCORES, nblk, BLK, NPG, DH)
                 .transpose(0, 1, 3, 2, 4)
                 .reshape(NCORES, nblk, NPG, BLK * DH)).copy()

    # per-edge local pair index: i = dst local, j = src local
    src = edge_index[0].astype(np.int64)
    dst = edge_index[1].astype(np.int64)
    pair = (dst % NPG) * NPG + (src % NPG)                 # [E]
    pairs = pair.reshape(G, EPG).astype(np.int16)          # per graph
    # wrapped [16, 18] per graph, replicated to both 16-channel groups
    wrapped = pairs.reshape(G, EPG // 16, 16).transpose(0, 2, 1)   # [G,16,18]
    idx16 = np.repeat(wrapped.reshape(G, 1, 16, EPG // 16), 2, axis=1)  # [G,2grp,16,18]
    idx16 = idx16.reshape(NCORES, nblk, 128, EPG // 16).copy()

    kk = np.arange(128) % K
    inv_col = inv[kk][:, None].astype(f32).copy()
    nmi_col = (-gbf_means[kk] * inv[kk])[:, None].astype(f32).copy()
    sel32 = np.zeros((32, 8 * 128), f32)
    for r in range(8):
        for g in range(BLK):
            sel32[4 * r + g, r * 128 + g * K:r * 128 + (g + 1) * K] = 1.0
    sel4 = np.tile(sel32, (4, 1))
    biasE = np.tile(b_edges.astype(f32), 12)[None, :].repeat(96, axis=0).copy()
    WcBD = np.zeros((128, BLK * DH), f32)
    We2BD = np.zeros((128, BLK * DH), f32)
    for g in range(BLK):
        WcBD[g * K:(g + 1) * K, g * DH:(g + 1) * DH] = Wc
        We2BD[g * K:(g + 1) * K, g * DH:(g + 1) * DH] = We2

    in_maps = []
    for c in range(NCORES):
        gsl = slice(c * gc, (c + 1) * gc)
        nsl = slice(c * gc * NPG, (c + 1) * gc * NPG)
        in_maps.append({
            "pos_in": np.ascontiguousarray(pos[nsl]).astype(f32),
            "mulT_in": np.ascontiguousarray(mulT[gsl]),
            "biasT_in": np.ascontiguousarray(biasT[gsl]),
            "aembR_in": np.ascontiguousarray(aembR[c]),
            "idx_in": np.ascontiguousarray(idx16[c]),
            "invc_in": inv_col, "nmic_in": nmi_col, "sel4_in": sel4,
            "Wc_in": WcBD, "We2_in": We2BD, "biasE_in": biasE,
        })
    return in_maps


# ---------------------------------------------------------------- entry
def kernel(z, pos, batch_mapping, edge_index, atom_emb, W_edge_proj,
           b_edge_proj, gbf_means, gbf_stds, gbf_mul, gbf_bias,
           W_nodes, b_nodes, W_edges, b_edges, _trace=False):
    z = np.asarray(z); pos = np.asarray(pos, np.float32)
    batch_mapping = np.asarray(batch_mapping)
    edge_index = np.asarray(edge_index)

    ok = (z.shape[0] == N and edge_index.shape == (2, E)
          and np.array_equal(batch_mapping, np.arange(N) // NPG)
          and bool((edge_index // NPG == np.arange(E) // EPG).all()))
    if not ok:
        return _numpy_reference(z, pos, batch_mapping, edge_index, atom_emb,
                                W_edge_proj, b_edge_proj, gbf_means, gbf_stds,
                                gbf_mul, gbf_bias, W_nodes, b_nodes, W_edges,
                                b_edges)

    from concourse.bass_utils import run_bass_kernel_spmd

    if GC not in _PROG_CACHE:
        _PROG_CACHE[GC] = _build_program(GC)
    nc = _PROG_CACHE[GC]

    in_maps = _host_prep(z, pos, edge_index, atom_emb, W_edge_proj,
                         b_edge_proj, gbf_means, gbf_stds, gbf_mul, gbf_bias,
                         W_nodes, b_nodes, W_edges, b_edges, GC)

    res = run_bass_kernel_spmd(nc, in_maps, core_ids=list(range(NCORES)),
                               trace=_trace)
    outs = res.results
    node_feat = np.concatenate([outs[c]["node_out"] for c in range(NCORES)], 0)
    edge_feat = np.concatenate([outs[c]["edge_out"] for c in range(NCORES)], 0)
    if _trace:
        kernel._last_results = res
    return node_feat.astype(np.float32), edge_feat.astype(np.float32)
